# revision 15
# baseline (speedup 1.0000x reference)
"""Trainium2 Bass kernel for nn_Net_51445118271652 (GNN graph matching).

Self-contained: builds an 8-core SPMD Bass program, shards inputs on the
host, runs via run_bass_kernel_spmd, reassembles full outputs.

Structure exploited:
  - layer-0 softmax/sinkhorn is dead code (s overwritten before use)
  - s is block-diagonal: two independent 2000x2000 blocks (nodes/edges)
  - sinkhorn = alternating reciprocal matvecs against the fixed base
    matrix t0; only the final s is materialized
  - column-normalized A folds into a row scaling of relu(x@Wa)

Sharding: rows of the 4000-dim (nodes+edges) across 8 cores; cores 0-3
own the node s-block, 4-7 the edge block.  Per-core divergence is
handled via host-sliced inputs plus a few partition_id-driven dynamic
DMA offsets; the SPMD program is identical on all cores.
"""
import sys

import numpy as np

sys.path.insert(0, "/opt/trn_rl_repo")

import concourse.bass as bass
import concourse.mybir as mybir
import concourse.tile as tile
from concourse.masks import make_identity
from concourse.bass_utils import run_bass_kernel_spmd
from concourse.vector_clock import ScopedClock

F32 = mybir.dt.float32
AF = mybir.ActivationFunctionType
ALU = mybir.AluOpType
AX = mybir.AxisListType

# ---------------------------------------------------------------- tile patch
# This walrus build accepts at most ONE sync wait per instruction; Tile's
# scheduler attaches more.  Hoist extras onto nofuse NOPs on the same engine.
_MAXW = 1
_REAL_TCW = tile.TileClockWait


def _split_excess_waits(nc, ordered):
    for bb_name, insts in ordered.items():
        new_list = []
        for inst in insts:
            si = inst.sync_info
            if si is not None and si.on_wait and len(si.on_wait) > _MAXW:
                waits = list(si.on_wait)
                si.on_wait = waits[-_MAXW:]
                for i in range(0, len(waits) - _MAXW, _MAXW):
                    new_list.append(mybir.InstNoOp(
                        name=nc.get_next_instruction_name(),
                        engine=inst.engine,
                        sync_info=mybir.SyncInfo(
                            on_wait=waits[i:i + _MAXW], on_update=[]),
                        bass_nofuse=True,
                        text_hint="waitsplit",
                    ))
            new_list.append(inst)
        ordered[bb_name] = new_list


class _SplitWaitTCW:
    def __init__(self, tc, ordered, **kw):
        object.__setattr__(self, "_real", _REAL_TCW(tc, ordered, **kw))
        object.__setattr__(self, "_tc", tc)
        object.__setattr__(self, "_ordered", ordered)

    def __getattr__(self, k):
        return getattr(object.__getattribute__(self, "_real"), k)

    def assign_waits(self, start_bb):
        r = object.__getattribute__(self, "_real").assign_waits(start_bb)
        _split_excess_waits(object.__getattribute__(self, "_tc").nc,
                            object.__getattribute__(self, "_ordered"))
        return r


def _patched_drain_and_barrier(self, tick_clock, wait_clock):
    nc = self.nc
    probe = nc.sync.nop(nofuse=True, hint="tail_wait_probe")
    wait_clock.add_sem_waits(probe.ins, ScopedClock({None: tick_clock.global_clock}))
    waits = (list(probe.ins.sync_info.on_wait)
             if probe.ins.sync_info and probe.ins.sync_info.on_wait else [])
    if len(waits) > _MAXW:
        probe.ins.sync_info.on_wait = waits[:_MAXW]
        rest = waits[_MAXW:]
        for i in range(0, len(rest), _MAXW):
            nop = nc.sync.nop(nofuse=True, hint=f"tail_wait_{i}")
            nop.ins.sync_info = mybir.SyncInfo(on_wait=rest[i:i + _MAXW], on_update=[])
    nc.sync.drain()
    nc.all_engine_barrier()
    popped = nc._tile_sem_poison_stack.pop()
    assert popped is self._sem_poison
    nc.clear_and_free_semaphores(list(self.sems.allocated().values()))
    nc.all_engine_barrier()


if tile.TileClockWait is not _SplitWaitTCW:
    tile.TileClockWait = _SplitWaitTCW
    tile.TileContext._drain_and_barrier = _patched_drain_and_barrier

# ---------------------------------------------------------------- constants
P = 128
N = 2000
NT = 4000
NTP = 4096
D = 128
H = 512
F = 256
L = 3
ALPHA = 200.0
EPS = 1e-4
NC = 8
MY = NT // NC     # 500
KP = N // NC      # 250

DEBUG = False


def _chunks(total, size=P):
    out = []
    s = 0
    while s < total:
        out.append((s, min(size, total - s)))
        s += size
    return out


C500 = _chunks(MY)
C2000 = _chunks(N)
C250 = _chunks(KP)
G2000 = _chunks(N, 512)
NK = NTP // P


class Prog:
    pass


def _mm(nc, psum_ap, pairs):
    n = len(pairs)
    for i, (lt, rt) in enumerate(pairs):
        nc.tensor.matmul(psum_ap, lt, rt, start=(i == 0), stop=(i == n - 1))


def _tr(nc, out_ap, in_ap, ident):
    k = in_ap.shape[0]
    nc.tensor.transpose(out_ap, in_ap, ident[:k, :k])


def _load_kchunks(nc, dst, src, nch, width):
    for k in range(nch):
        nc.sync.dma_start(dst[:, k * width:(k + 1) * width],
                          src[k * P:(k + 1) * P, :])


def build_program():
    pg = Prog()
    nc = bass.Bass(num_devices=NC)
    pg.nc = nc

    def din(name, shape):
        setattr(pg, name, nc.dram_tensor(name, list(shape), F32,
                                         kind="ExternalInput"))

    def dout(name, shape):
        setattr(pg, name, nc.dram_tensor(name, list(shape), F32,
                                         kind="ExternalOutput"))

    for nm in ("emb1", "emb2", "eed1", "eed2"):
        din(nm, (N, D))
    din("emb1m", (KP, D))
    din("eed1m", (KP, D))
    din("a1s", (MY, NTP))
    din("a2s", (MY, NTP))
    din("w1n", (D, H)); din("w2n", (H, F))
    din("w1e", (D, H)); din("w2e", (H, F))
    din("wa", (L, F, F)); din("wu", (L, F, F)); din("ssym", (L, F, F))
    din("cgw", (2 * F, F))

    dout("s_out", (MY, N))
    dout("kp", (KP, N))
    dout("ke", (KP, N))
    if DEBUG:
        for i in range(L):
            dout(f"d_x1m_{i}", (2 * P, MY))
            dout(f"d_x2m_{i}", (2 * P, MY))
        for i in (1, 2):
            dout(f"d_t0_{i}", (MY, N))

    with tile.TileContext(nc) as tc:
        _emit(pg, tc)
    return pg


def _emit(pg, tc):
    nc = pg.nc
    pg._cms = cms = []

    def open_pool(**kw):
        cm = tc.tile_pool(**kw)
        cms.append(cm)
        return cm.__enter__()

    const = open_pool(name="const", bufs=1)
    dram = open_pool(name="dram", bufs=1, space="DRAM")
    drb = open_pool(name="drb", bufs=2, space="DRAM")

    ident = const.tile([P, P], F32)
    make_identity(nc, ident)
    ones_r = const.tile([1, P], F32)
    nc.vector.memset(ones_r[:], 1.0)
    ones_c = const.tile([P, 1], F32)
    nc.vector.memset(ones_c[:], 1.0)

    pid = nc.sync.partition_id()
    base_my = pid * MY
    base_blk = (pid // 4) * N
    base_row = (pid % 4) * MY
    pg._base_row = base_row
    pg._base_blk = base_blk

    big = open_pool(name="big", bufs=1)
    x1Tm = [big.tile([P, MY], F32, name=f"x1Tm{c}") for c in range(2)]
    x2Tm = [big.tile([P, MY], F32, name=f"x2Tm{c}") for c in range(2)]
    t0 = [big.tile([ln, N], F32, name=f"t0_{i}", padded_shape=[P, N])
          for i, (st, ln) in enumerate(C500)]
    invc1 = big.tile([P, NK], F32, name="invc1")
    invc2 = big.tile([P, NK], F32, name="invc2")

    a1t = dram.tile([NTP, MY], F32, name="a1t")
    a2t = dram.tile([NTP, MY], F32, name="a2t")
    # xt_d rows: (xsel*2 + fc)*128 ; embeddings output, layer-0 input
    xt_d = dram.tile([4 * P, NTP], F32, name="xt_d")
    cvec_d = dram.tile([1, 2048], F32, name="cvec_d")
    rvec_d = dram.tile([1, 512], F32, name="rvec_d")
    pg._cvec_d = cvec_d
    pg._rvec_d = rvec_d

    # ================= embeddings -> xt_d (replicated) ===================
    e1nmT = [const.tile([P, KP], F32, name=f"e1nmT{c}") for c in range(2)]
    e1emT = [const.tile([P, KP], F32, name=f"e1emT{c}") for c in range(2)]
    kp_x2 = const.tile([P, 4], F32, name="kp_x2")

    with tc.tile_pool(name="wemb", bufs=1) as wp, \
         tc.tile_pool(name="embwork", bufs=2) as ew, \
         tc.tile_pool(name="psE", bufs=2, space="PSUM") as psE:
        w1n_t = wp.tile([P, H], F32)
        nc.sync.dma_start(w1n_t[:], pg.w1n[:, :])
        w1e_t = wp.tile([P, H], F32)
        nc.sync.dma_start(w1e_t[:], pg.w1e[:, :])
        w2n_t = wp.tile([P, 4 * F], F32)
        _load_kchunks(nc, w2n_t, pg.w2n, 4, F)
        w2e_t = wp.tile([P, 4 * F], F32)
        _load_kchunks(nc, w2e_t, pg.w2e, 4, F)

        def pipe(embs, w1t, w2t, nonlin, writer, rows, tagp):
            CR = _chunks(rows)
            embT = ew.tile([P, rows], F32, tag="embT", bufs=1,
                           name=f"embT{tagp}")
            for ci, (st, ln) in enumerate(CR):
                cht = ew.tile([ln, D], F32, tag="embch", name=f"ec{tagp}{ci}",
                              padded_shape=[P, D], bufs=3)
                nc.sync.dma_start(cht[:], embs[st:st + ln, :])
                pt = psE.tile([P, ln], F32, tag="pt", name=f"ept{tagp}{ci}",
                              padded_shape=[P, P])
                _tr(nc, pt[:], cht[:], ident)
                nc.vector.tensor_copy(embT[:, st:st + ln], pt[:])
            h1T = ew.tile([P, 4 * rows], F32, tag="h1T", bufs=1,
                          name=f"h1T{tagp}")
            for m in range(4):
                for gs, gl in _chunks(rows, 512):
                    ph = psE.tile([P, 512], F32, tag="eph",
                                  name=f"eph{tagp}{m}{gs}")
                    _mm(nc, ph[:, :gl], [(w1t[:, m * P:(m + 1) * P],
                                          embT[:, gs:gs + gl])])
                    nc.scalar.activation(h1T[:, m * rows + gs: m * rows + gs + gl],
                                         ph[:, :gl], AF.Relu)
            for ci, (st, ln) in enumerate(CR):
                pz = psE.tile([P, F], F32, tag="epz", name=f"epz{tagp}{ci}")
                _mm(nc, pz[:ln, :], [(h1T[:, k * rows + st: k * rows + st + ln],
                                      w2t[:, k * F:(k + 1) * F]) for k in range(4)])
                z = ew.tile([ln, F], F32, tag="zch", name=f"ez{tagp}{ci}",
                            padded_shape=[P, F], bufs=3)
                if nonlin == "sig":
                    nc.scalar.activation(z[:], pz[:ln, :], AF.Sigmoid)
                else:
                    nc.vector.tensor_copy(z[:], pz[:ln, :])
                    scr = ew.tile([ln, F], F32, tag="scr", name=f"es{tagp}{ci}",
                                  padded_shape=[P, F], bufs=3)
                    ssq = ew.tile([ln, 1], F32, tag="ssq", name=f"eq{tagp}{ci}",
                                  padded_shape=[P, 1], bufs=3)
                    nc.vector.tensor_tensor(scr[:], z[:], z[:], op=ALU.mult)
                    nc.vector.reduce_sum(ssq[:], scr[:], axis=AX.X)
                    nc.scalar.activation(ssq[:], ssq[:], AF.Sqrt)
                    nc.vector.tensor_scalar_max(ssq[:], ssq[:], 1e-12)
                    nc.vector.reciprocal(ssq[:], ssq[:])
                    nc.vector.tensor_scalar_mul(z[:], z[:], ssq[:])
                writer(ci, st, ln, z)

        def write_xt_d(xsel, coloff, tagp):
            def w(ci, st, ln, z):
                for fc in range(2):
                    pt = psE.tile([P, ln], F32, tag="pt", name=f"wx{tagp}{ci}{fc}",
                                  padded_shape=[P, P])
                    _tr(nc, pt[:], z[:, fc * P:(fc + 1) * P], ident)
                    stg = ew.tile([P, ln], F32, tag="wstg",
                                  name=f"ws{tagp}{ci}{fc}",
                                  padded_shape=[P, P], bufs=3)
                    nc.vector.tensor_copy(stg[:], pt[:])
                    nc.sync.dma_start(
                        xt_d[(xsel * 2 + fc) * P:(xsel * 2 + fc + 1) * P,
                             coloff + st: coloff + st + ln], stg[:])
            return w

        def write_mT(dst, x2col, tagp):
            def w(ci, st, ln, z):
                for fc in range(2):
                    pt = psE.tile([P, ln], F32, tag="pt", name=f"wm{tagp}{ci}{fc}",
                                  padded_shape=[P, P])
                    _tr(nc, pt[:], z[:, fc * P:(fc + 1) * P], ident)
                    nc.vector.tensor_copy(dst[fc][:, st:st + ln], pt[:])
                scr2 = ew.tile([ln, F], F32, tag="scr", name=f"k2{tagp}{ci}",
                               padded_shape=[P, F], bufs=3)
                nc.vector.tensor_tensor(scr2[:], z[:], z[:], op=ALU.mult)
                nc.vector.reduce_sum(kp_x2[:ln, x2col * 2 + ci: x2col * 2 + ci + 1],
                                     scr2[:], axis=AX.X)
            return w

        pipe(pg.emb1, w1n_t, w2n_t, "l2", write_xt_d(0, 0, "a"), N, "a")
        pipe(pg.emb2, w1n_t, w2n_t, "l2", write_xt_d(1, 0, "b"), N, "b")
        pipe(pg.eed1, w1e_t, w2e_t, "sig", write_xt_d(0, N, "c"), N, "c")
        pipe(pg.eed2, w1e_t, w2e_t, "sig", write_xt_d(1, N, "d"), N, "d")
        pipe(pg.emb1m, w1n_t, w2n_t, "l2", write_mT(e1nmT, 0, "e"), KP, "e")
        pipe(pg.eed1m, w1e_t, w2e_t, "sig", write_mT(e1emT, 1, "f"), KP, "f")
        zpad = ew.tile([P, NTP - NT], F32, tag="zpad", bufs=1, name="zpad")
        nc.vector.memset(zpad[:], 0.0)
        for rb_ in range(4):
            nc.sync.dma_start(xt_d[rb_ * P:(rb_ + 1) * P, NT:NTP], zpad[:])

    # initial mine slices (dynamic from DRAM)
    for c in range(2):
        nc.sync.dma_start(x1Tm[c][:],
                          xt_d[c * P:(c + 1) * P, bass.ds(base_my, MY)])
        nc.sync.dma_start(x2Tm[c][:],
                          xt_d[(2 + c) * P:(2 + c + 1) * P, bass.ds(base_my, MY)])

    # ================= K matrices =======================================
    with tc.tile_pool(name="kwork", bufs=1) as kw, \
         tc.tile_pool(name="psK", bufs=2, space="PSUM") as psK:
        x2Tk = [kw.tile([P, NT], F32, name=f"x2Tk{c}") for c in range(2)]
        for c in range(2):
            nc.sync.dma_start(x2Tk[c][:], xt_d[(2 + c) * P:(2 + c + 1) * P, 0:NT])
        y2b = {}
        scrK = [kw.tile([P, N], F32, name=f"scrK{fc}") for fc in range(2)]
        for which, off in (("kp", 0), ("ke", N)):
            for fc in range(2):
                nc.vector.tensor_tensor(scrK[fc][:], x2Tk[fc][:, off:off + N],
                                        x2Tk[fc][:, off:off + N], op=ALU.mult)
            y2v = kw.tile([1, N], F32, name=f"y2v{which}")
            for gs, gl in G2000:
                psy = psK.tile([1, 512], F32, tag="psy", name=f"psy{which}{gs}", bufs=1)
                for fc in range(2):
                    nc.tensor.matmul(psy[:, :gl], ones_c[:],
                                     scrK[fc][:, gs:gs + gl],
                                     start=(fc == 0), stop=(fc == 1))
                nc.scalar.copy(y2v[:, gs:gs + gl], psy[:, :gl])
            y2 = kw.tile([P, N], F32, name=f"y2b{which}")
            for gs, gl in G2000:
                pb = psK.tile([P, 512], F32, tag="pby", name=f"pby{which}{gs}", bufs=1)
                nc.tensor.matmul(pb[:, :gl], ones_r[:], y2v[:, gs:gs + gl],
                                 start=True, stop=True)
                nc.scalar.copy(y2[:, gs:gs + gl], pb[:, :gl])
            y2b[which] = y2

        kmax_p = kw.tile([1, 2], F32, name="kmax_p")
        Gt = {}
        for which, emT, x2off, xcol in (("kp", e1nmT, 0, 0), ("ke", e1emT, N, 1)):
            G = [kw.tile([ln, N], F32, name=f"G{which}{ci}", padded_shape=[P, N])
                 for ci, (st, ln) in enumerate(C250)]
            Gt[which] = G
            rmax = kw.tile([P, 2], F32, name=f"rmax{which}")
            nc.vector.memset(rmax[:], 0.0)
            for ci, (st, ln) in enumerate(C250):
                g = G[ci]
                for gs, gl in G2000:
                    pg_ = psK.tile([P, 512], F32, tag="gp",
                                   name=f"Gp{which}{ci}{gs}", bufs=3)
                    _mm(nc, pg_[:ln, :gl],
                        [(emT[fc][:, st:st + ln],
                          x2Tk[fc][:, x2off + gs:x2off + gs + gl])
                         for fc in range(2)])
                    nc.scalar.copy(g[:, gs:gs + gl], pg_[:ln, :gl])
                nc.vector.tensor_scalar(
                    g[:], g[:], -2.0, kp_x2[:ln, xcol * 2 + ci: xcol * 2 + ci + 1],
                    op0=ALU.mult, op1=ALU.add)
                nc.vector.tensor_tensor(g[:], g[:], y2b[which][:ln, :], op=ALU.add)
                nc.vector.tensor_scalar_max(g[:], g[:], 0.0)
                nc.scalar.activation(g[:], g[:], AF.Sqrt)
                nc.vector.reduce_max(rmax[:ln, ci:ci + 1], g[:], axis=AX.X)
            mm2 = kw.tile([P, 1], F32, name=f"mm2{which}")
            nc.vector.tensor_tensor(mm2[:], rmax[:, 0:1], rmax[:, 1:2], op=ALU.max)
            col = 0 if which == "kp" else 1
            nc.gpsimd.tensor_reduce(kmax_p[0:1, col:col + 1], mm2[:],
                                    axis=AX.C, op=ALU.max)
        kin = drb.tile([1, 2], F32, name="kin")
        kout = drb.tile([1, 2], F32, name="kout", addr_space="Shared")
        nc.sync.dma_start(kin[:], kmax_p[:])
        nc.gpsimd.collective_compute(
            "AllReduce", ALU.max, ins=[kin.opt()], outs=[kout.opt()],
            replica_groups=[list(range(NC))])
        kmax = kw.tile([1, 2], F32, name="kmax")
        nc.sync.dma_start(kmax[:], kout[:])
        kinv = kw.tile([1, 2], F32, name="kinv")
        nc.vector.reciprocal(kinv[:], kmax[:])
        pkb = psK.tile([P, 2], F32, tag="pkb", name="pkb", bufs=1)
        nc.tensor.matmul(pkb[:], ones_r[:], kinv[:], start=True, stop=True)
        kinvb = kw.tile([P, 2], F32, name="kinvb")
        nc.scalar.copy(kinvb[:], pkb[:])
        for which, out in (("kp", pg.kp), ("ke", pg.ke)):
            col = 0 if which == "kp" else 1
            for ci, (st, ln) in enumerate(C250):
                g = Gt[which][ci]
                nc.vector.tensor_scalar(g[:], g[:], kinvb[:ln, col:col + 1], -1.0,
                                        op0=ALU.mult, op1=ALU.mult)
                nc.vector.tensor_scalar_add(g[:], g[:], 1.0)
                nc.sync.dma_start(out[st:st + ln, :], g[:])

    # ================= A transpose + invc ===============================
    with tc.tile_pool(name="atw", bufs=1) as atw, \
         tc.tile_pool(name="psA", bufs=4, space="PSUM") as psA:
        for (asrc, adst, invc, bn) in ((pg.a1s, a1t, invc1, "A"),
                                       (pg.a2s, a2t, invc2, "B")):
            for ri, (rs, rl) in enumerate(C500):
                nat = atw.tile([rl, NTP], F32, tag="anat", name=f"an{bn}{ri}",
                               padded_shape=[P, NTP], bufs=2)
                nc.sync.dma_start(nat[:], asrc[rs:rs + rl, :])
                stg = atw.tile([P, NK * rl], F32, tag="astg",
                               name=f"as{bn}{ri}", padded_shape=[P, NK * P],
                               bufs=2)
                for jc in range(NK):
                    ptt = psA.tile([P, rl], F32, tag="pt", name=f"at{bn}{ri}{jc}",
                                   padded_shape=[P, P])
                    _tr(nc, ptt[:], nat[:, jc * P:(jc + 1) * P], ident)
                    if jc % 2 == 0:
                        nc.vector.tensor_copy(stg[:, jc * rl:(jc + 1) * rl],
                                              ptt[:])
                    else:
                        nc.scalar.copy(stg[:, jc * rl:(jc + 1) * rl], ptt[:])
                for jc in range(NK):
                    nc.sync.dma_start(adst[jc * P:(jc + 1) * P, rs:rs + rl],
                                      stg[:, jc * rl:(jc + 1) * rl])
            for jc in range(NK):
                ch = atw.tile([P, MY], F32, tag="acs", name=f"ac{bn}{jc}",
                              bufs=4)
                nc.sync.dma_start(ch[:], adst[jc * P:(jc + 1) * P, :])
                nc.vector.tensor_reduce(invc[:, jc:jc + 1], ch[:], axis=AX.X,
                                        op=ALU.add)
        ain = drb.tile([P, 2 * NK], F32, name="ain")
        aout = drb.tile([P, 2 * NK], F32, name="aout", addr_space="Shared")
        nc.sync.dma_start(ain[:, 0:NK], invc1[:])
        nc.sync.dma_start(ain[:, NK:2 * NK], invc2[:])
        nc.gpsimd.collective_compute(
            "AllReduce", ALU.add, ins=[ain.opt()], outs=[aout.opt()],
            replica_groups=[list(range(NC))])
        nc.sync.dma_start(invc1[:], aout[:, 0:NK])
        nc.sync.dma_start(invc2[:], aout[:, NK:2 * NK])
        for invc in (invc1, invc2):
            nc.vector.tensor_scalar_max(invc[:], invc[:], 1e-12)
            nc.vector.reciprocal(invc[:], invc[:])

    # ================= GNN layers =======================================
    lw = open_pool(name="lw", bufs=1)
    ago_prev = [None]

    for layer in range(L):
        wa_t = lw.tile([P, 2 * F], F32, tag="wa", name=f"wa{layer}")
        _load_kchunks(nc, wa_t, pg.wa[layer], 2, F)
        wu_t = lw.tile([P, 2 * F], F32, tag="wu", name=f"wu{layer}")
        _load_kchunks(nc, wu_t, pg.wu[layer], 2, F)

        with tc.tile_pool(name=f"gw{layer}", bufs=1) as gw:
            x2Tb = [gw.tile([P, N], F32, tag=f"x2Tb{c}", name=f"x2Tb{layer}{c}")
                    for c in range(2)]
            # ---- gconv phase (full xT resident only here) ---------------
            with tc.tile_pool(name=f"xTp{layer}", bufs=1) as xp, \
                 tc.tile_pool(name=f"psG{layer}", bufs=1, space="PSUM") as psG:
                x1T = [xp.tile([P, NTP], F32, name=f"x1T{layer}{c}")
                       for c in range(2)]
                x2T = [xp.tile([P, NTP], F32, name=f"x2T{layer}{c}")
                       for c in range(2)]
                if ago_prev[0] is None:
                    for g, xT in ((0, x1T), (1, x2T)):
                        for c in range(2):
                            nc.sync.dma_start(
                                xT[c][:],
                                xt_d[(g * 2 + c) * P:(g * 2 + c + 1) * P, :])
                else:
                    ago = ago_prev[0]
                    for r in range(NC):
                        for g, xT in ((0, x1T), (1, x2T)):
                            for c in range(2):
                                src = ago[(r * 4 + g * 2 + c) * P:
                                          (r * 4 + g * 2 + c + 1) * P, :]
                                nc.sync.dma_start(
                                    xT[c][:, r * MY:(r + 1) * MY], src)
                    for t in (x1T[0], x1T[1], x2T[0], x2T[1]):
                        nc.vector.memset(t[:, NT:NTP], 0.0)

                for (xT, xTm, at_d, invc, gname) in (
                        (x1T, x1Tm, a1t, invc1, "g1"),
                        (x2T, x2Tm, a2t, invc2, "g2")):
                    pA = [psG.tile([P, MY], F32, tag=f"pA{m}",
                                   name=f"pA{layer}{gname}{m}", bufs=1)
                          for m in range(2)]
                    uxT = gw.tile([P, 2 * MY], F32, tag="uxT",
                                  name=f"uxT{layer}{gname}")
                    for m in range(2):
                        pU = psG.tile([P, MY], F32, tag="pU",
                                      name=f"pU{layer}{gname}{m}", bufs=1)
                        _mm(nc, pU[:],
                            [(wu_t[:, fc * F + m * P: fc * F + (m + 1) * P],
                              xTm[fc][:]) for fc in range(2)])
                        nc.scalar.activation(uxT[:, m * MY:(m + 1) * MY], pU[:],
                                             AF.Relu)
                    for kc in range(NK):
                        py = psG.tile([P, F], F32, tag="py",
                                      name=f"py{layer}{gname}{kc}",
                                      padded_shape=[P, 512], bufs=3)
                        _mm(nc, py[:], [(xT[fc][:, kc * P:(kc + 1) * P],
                                         wa_t[:, fc * F:(fc + 1) * F])
                                        for fc in range(2)])
                        ych = gw.tile([P, F], F32, tag="ych",
                                      name=f"y{layer}{gname}{kc}", bufs=3)
                        nc.scalar.activation(ych[:], py[:], AF.Relu,
                                             scale=invc[:, kc:kc + 1])
                        atch = gw.tile([P, MY], F32, tag="atch",
                                       name=f"at{layer}{gname}{kc}", bufs=3)
                        nc.sync.dma_start(atch[:], at_d[kc * P:(kc + 1) * P, :])
                        for m in range(2):
                            nc.tensor.matmul(pA[m][:], ych[:, m * P:(m + 1) * P],
                                             atch[:], start=(kc == 0),
                                             stop=(kc == NK - 1))
                    xnT = gw.tile([P, 2 * MY], F32, tag="xnT",
                                  name=f"xnT{layer}{gname}")
                    for m in range(2):
                        nc.vector.tensor_tensor(xnT[:, m * MY:(m + 1) * MY],
                                                pA[m][:],
                                                uxT[:, m * MY:(m + 1) * MY],
                                                op=ALU.add)
                    scr = gw.tile([P, MY], F32, tag="gscr",
                                  name=f"gscr{layer}{gname}", bufs=2)
                    pn = psG.tile([1, 512], F32, tag="pn",
                                  name=f"pn{layer}{gname}", bufs=1)
                    for m in range(2):
                        nc.vector.tensor_tensor(scr[:],
                                                xnT[:, m * MY:(m + 1) * MY],
                                                xnT[:, m * MY:(m + 1) * MY],
                                                op=ALU.mult)
                        nc.tensor.matmul(pn[:, :MY], ones_c[:], scr[:],
                                         start=(m == 0), stop=(m == 1))
                    nrm = gw.tile([1, MY], F32, tag="gnrm",
                                  name=f"gn{layer}{gname}", bufs=2)
                    nc.scalar.activation(nrm[:], pn[:, :MY], AF.Sqrt)
                    nc.vector.tensor_scalar_max(nrm[:], nrm[:], 1e-12)
                    nc.vector.reciprocal(nrm[:], nrm[:])
                    pnb = psG.tile([P, 512], F32, tag="pnb",
                                   name=f"pnb{layer}{gname}", bufs=1)
                    nc.tensor.matmul(pnb[:, :MY], ones_r[:], nrm[:],
                                     start=True, stop=True)
                    nrb = gw.tile([P, MY], F32, tag="gnrb",
                                  name=f"gnb{layer}{gname}", bufs=2)
                    nc.scalar.copy(nrb[:], pnb[:, :MY])
                    for m in range(2):
                        nc.vector.tensor_tensor(xTm[m][:],
                                                xnT[:, m * MY:(m + 1) * MY],
                                                nrb[:], op=ALU.mult)

                ago_prev[0] = _allgather_xT(pg, nc, drb, x1Tm, x2Tm)
                if layer > 0:
                    # group-local allgather of x2Tm -> my 2000-col x2 block
                    x2gi = drb.tile([2 * P, MY], F32, tag="x2gi",
                                    name=f"x2gi{layer}")
                    x2go = drb.tile([4 * 2 * P, MY], F32, tag="x2go",
                                    name=f"x2go{layer}")
                    for c in range(2):
                        nc.sync.dma_start(x2gi[c * P:(c + 1) * P, :], x2Tm[c][:])
                    nc.gpsimd.collective_compute(
                        "AllGather", ALU.bypass, ins=[x2gi.opt()],
                        outs=[x2go.opt()],
                        replica_groups=[[0, 1, 2, 3], [4, 5, 6, 7]])
                    for k in range(4):
                        for c in range(2):
                            nc.sync.dma_start(
                                x2Tb[c][:, k * MY:(k + 1) * MY],
                                x2go[(k * 2 + c) * P:(k * 2 + c + 1) * P, :])

            if DEBUG:
                for c in range(2):
                    nc.sync.dma_start(
                        getattr(pg, f"d_x1m_{layer}")[c * P:(c + 1) * P, :],
                        x1Tm[c][:])
                    nc.sync.dma_start(
                        getattr(pg, f"d_x2m_{layer}")[c * P:(c + 1) * P, :],
                        x2Tm[c][:])

            if layer == 0:
                continue

            # ---------- affinity + softmax -> t0 -------------------------
            ss_t = lw.tile([P, 2 * F], F32, tag="ss", name=f"ss{layer}")
            _load_kchunks(nc, ss_t, pg.ssym[layer], 2, F)
            with tc.tile_pool(name=f"psF{layer}", bufs=1, space="PSUM") as psF:
                z1T = gw.tile([P, 2 * MY], F32, tag="z1T", name=f"z1T{layer}")
                for ci, (st, ln) in enumerate(C500):
                    pz = psF.tile([P, F], F32, tag="pz1", name=f"pz1{layer}{ci}",
                                  bufs=2)
                    _mm(nc, pz[:ln, :], [(x1Tm[fc][:, st:st + ln],
                                          ss_t[:, fc * F:(fc + 1) * F])
                                         for fc in range(2)])
                    z1 = gw.tile([ln, F], F32, tag="z1", name=f"z1{layer}{ci}",
                                 padded_shape=[P, F], bufs=2)
                    nc.scalar.copy(z1[:], pz[:ln, :])
                    for fc in range(2):
                        ptz = psF.tile([P, ln], F32, tag="pt",
                                       name=f"ptz{layer}{ci}{fc}",
                                       padded_shape=[P, P], bufs=2)
                        _tr(nc, ptz[:], z1[:, fc * P:(fc + 1) * P], ident)
                        nc.vector.tensor_copy(
                            z1T[:, fc * MY + st: fc * MY + st + ln], ptz[:])
                for ci, (st, ln) in enumerate(C500):
                    tt = t0[ci]
                    for gs, gl in G2000:
                        ps_ = psF.tile([P, 512], F32, tag="psf",
                                       name=f"psf{layer}{ci}{gs}", bufs=3)
                        _mm(nc, ps_[:ln, :gl],
                            [(z1T[:, fc * MY + st: fc * MY + st + ln],
                              x2Tb[fc][:, gs:gs + gl]) for fc in range(2)])
                        nc.scalar.copy(tt[:, gs:gs + gl], ps_[:ln, :gl])
                    m0 = gw.tile([ln, 1], F32, tag="m0", name=f"m0{layer}{ci}",
                                 padded_shape=[P, 1], bufs=4)
                    nc.vector.reduce_max(m0[:], tt[:], axis=AX.X)
                    nc.vector.tensor_scalar_max(m0[:], m0[:], 0.0)
                    bm = gw.tile([ln, 1], F32, tag="bm", name=f"bm{layer}{ci}",
                                 padded_shape=[P, 1], bufs=4)
                    nc.vector.tensor_scalar_mul(bm[:], m0[:], -ALPHA)
                    nc.scalar.activation(tt[:], tt[:], AF.Exp, bias=bm[:],
                                         scale=ALPHA)
                    zz = gw.tile([ln, 1], F32, tag="zz", name=f"zz{layer}{ci}",
                                 padded_shape=[P, 1], bufs=4)
                    nc.vector.reduce_sum(zz[:], tt[:], axis=AX.X)
                    ee = gw.tile([ln, 1], F32, tag="ee", name=f"ee{layer}{ci}",
                                 padded_shape=[P, 1], bufs=4)
                    nc.scalar.activation(ee[:], bm[:], AF.Exp)
                    nc.vector.tensor_scalar(ee[:], ee[:], float(NT - N), zz[:],
                                            op0=ALU.mult, op1=ALU.add)
                    nc.vector.reciprocal(ee[:], ee[:])
                    nc.vector.tensor_scalar(tt[:], tt[:], ee[:], EPS,
                                            op0=ALU.mult, op1=ALU.add)
            if DEBUG:
                for ci, (st, ln) in enumerate(C500):
                    nc.sync.dma_start(getattr(pg, f"d_t0_{layer}")[st:st + ln, :],
                                      t0[ci][:])

            # ---------- t0T + sinkhorn -----------------------------------
            with tc.tile_pool(name=f"snk{layer}", bufs=1) as sp:
                t0T = [sp.tile([ln, MY], F32, name=f"t0T{layer}{j}",
                               padded_shape=[P, MY])
                       for j, (js, ln) in enumerate(C2000)]
                c_p = sp.tile([P, len(C2000)], F32, name=f"c_p{layer}")
                r_p = sp.tile([P, len(C500)], F32, name=f"r_p{layer}")
                cvec = sp.tile([1, N], F32, name=f"cvec{layer}")
                with tc.tile_pool(name=f"psN{layer}", bufs=1, space="PSUM") as psN:
                    for ci, (cs, cl) in enumerate(C500):
                        for j, (js, jl) in enumerate(C2000):
                            ptt = psN.tile([jl, cl], F32, tag="pt",
                                           name=f"pt0{layer}{ci}{j}",
                                           padded_shape=[P, P], bufs=3)
                            _tr(nc, ptt[:], t0[ci][:, js:js + jl], ident)
                            if (ci + j) % 2 == 0:
                                nc.vector.tensor_copy(t0T[j][:, cs:cs + cl],
                                                      ptt[:])
                            else:
                                nc.scalar.copy(t0T[j][:, cs:cs + cl], ptt[:])
                    nc.vector.memset(c_p[:], 1.0)
                    nc.vector.memset(r_p[:], 1.0)
                    for it in range(5):
                        pc = psN.tile([P, len(C2000)], F32, tag="pc",
                                      name=f"pc{layer}{it}", bufs=2)
                        for j, (js, jl) in enumerate(C2000):
                            for ci, (cs, cl) in enumerate(C500):
                                rhs = (ones_c[:cl, :] if it == 0
                                       else r_p[:cl, ci:ci + 1])
                                nc.tensor.matmul(pc[:jl, j:j + 1],
                                                 t0[ci][:, js:js + jl], rhs,
                                                 start=(ci == 0), stop=(ci == 3))
                        cin = drb.tile([P, len(C2000)], F32, tag="cin",
                                       name=f"ci{layer}{it}")
                        nc.vector.tensor_copy(c_p[:], pc[:])
                        nc.sync.dma_start(cin[:], c_p[:])
                        cout = drb.tile([P, len(C2000)], F32, tag="cout",
                                        name=f"co{layer}{it}")
                        nc.gpsimd.collective_compute(
                            "AllReduce", ALU.add, ins=[cin.opt()],
                            outs=[cout.opt()],
                            replica_groups=[[0, 1, 2, 3], [4, 5, 6, 7]])
                        nc.sync.dma_start(c_p[:], cout[:])
                        nc.vector.reciprocal(c_p[:], c_p[:])
                        pr = psN.tile([P, len(C500)], F32, tag="pr",
                                      name=f"pr{layer}{it}", bufs=2)
                        for ci, (cs, cl) in enumerate(C500):
                            for j, (js, jl) in enumerate(C2000):
                                nc.tensor.matmul(pr[:cl, ci:ci + 1],
                                                 t0T[j][:, cs:cs + cl],
                                                 c_p[:jl, j:j + 1],
                                                 start=(j == 0),
                                                 stop=(j == len(C2000) - 1))
                        nc.vector.tensor_copy(r_p[:], pr[:])
                        nc.vector.reciprocal(r_p[:], r_p[:])

                    ptc = psN.tile([len(C2000), P], F32, tag="pt",
                                   name=f"ptc{layer}", padded_shape=[P, P],
                                   bufs=3)
                    _tr(nc, ptc[:], c_p[:], ident)
                    ctr = sp.tile([len(C2000), P], F32, name=f"ctr{layer}",
                                  padded_shape=[P, P])
                    nc.vector.tensor_copy(ctr[:], ptc[:])
                    nc.sync.dma_start(
                        cvec_d[0:1, :].rearrange("o (c p) -> (o c) p", p=P),
                        ctr[:])
                    nc.sync.dma_start(cvec[:], cvec_d[0:1, 0:N])

                if layer == L - 2:
                    _cross_graph(pg, tc, nc, sp, lw, drb, t0, t0T, c_p, r_p,
                                 x1Tm, x2Tm, x2Tb, ident, ones_r)
                    ago_prev[0] = _allgather_xT(pg, nc, drb, x1Tm, x2Tm)

                if layer == L - 1:
                    with tc.tile_pool(name="psM", bufs=1, space="PSUM") as psM:
                        cb = sp.tile([P, N], F32, name="cb")
                        for gs, gl in G2000:
                            pcb = psM.tile([P, 512], F32, tag="pcb",
                                           name=f"pcb{gs}", bufs=2)
                            nc.tensor.matmul(pcb[:, :gl], ones_r[:],
                                             cvec[:, gs:gs + gl],
                                             start=True, stop=True)
                            nc.scalar.copy(cb[:, gs:gs + gl], pcb[:, :gl])
                        for ci, (cs, cl) in enumerate(C500):
                            nc.vector.tensor_scalar_mul(t0[ci][:], t0[ci][:],
                                                        r_p[:cl, ci:ci + 1])
                            nc.vector.tensor_tensor(t0[ci][:], t0[ci][:],
                                                    cb[:cl, :], op=ALU.mult)
                            nc.sync.dma_start(pg.s_out[cs:cs + cl, :], t0[ci][:])

    for cm in reversed(cms):
        cm.__exit__(None, None, None)


def _allgather_xT(pg, nc, drb, x1Tm, x2Tm):
    agi = drb.tile([4 * P, MY], F32, tag="agi", name=f"agi{nc.next_id()}")
    ago = drb.tile([NC * 4 * P, MY], F32, tag="ago", name=f"ago{nc.next_id()}",
                   addr_space="Shared")
    for g, xm in ((0, x1Tm), (1, x2Tm)):
        for c in range(2):
            nc.sync.dma_start(agi[(g * 2 + c) * P:(g * 2 + c + 1) * P, :], xm[c][:])
    nc.gpsimd.collective_compute(
        "AllGather", ALU.bypass, ins=[agi.opt()], outs=[ago.opt()],
        replica_groups=[list(range(NC))])
    return ago


def _cross_graph(pg, tc, nc, sp, lw, drb, t0, t0T, c_p, r_p,
                 x1Tm, x2Tm, x2Tb, ident, ones_r):
    cgw_t = lw.tile([P, 4 * F], F32, tag="cgw", name="cgw_t")
    _load_kchunks(nc, cgw_t, pg.cgw, 4, F)

    with tc.tile_pool(name="psX", bufs=1, space="PSUM") as psX:
        # r in true order -> rvec
        ptr = psX.tile([len(C500), P], F32, tag="pt", name="ptr",
                       padded_shape=[P, P], bufs=2)
        _tr(nc, ptr[:], r_p[:], ident)
        rtr = sp.tile([len(C500), P], F32, name="rtr", padded_shape=[P, P])
        nc.vector.tensor_copy(rtr[:], ptr[:])
        nc.sync.dma_start(
            pg._rvec_d[0:1, :].rearrange("o (c p) -> (o c) p", p=P), rtr[:])
        rvec = sp.tile([1, MY], F32, name="rvec")
        nc.sync.dma_start(rvec[:], pg._rvec_d[0:1, 0:MY])

        # ---- y1T = (w2^T @ t0^T) * r ; w2 = c * x2blk (natural)
        x2b = [sp.tile([ln, F], F32, name=f"x2b{j}", padded_shape=[P, F])
               for j, (js, ln) in enumerate(C2000)]
        for j, (js, jl) in enumerate(C2000):
            for fc in range(2):
                pt = psX.tile([jl, P], F32, tag="pt", name=f"px2b{j}{fc}",
                              padded_shape=[P, P], bufs=2)
                _tr(nc, pt[:], x2Tb[fc][:, js:js + jl], ident)
                nc.vector.tensor_copy(x2b[j][:, fc * P:(fc + 1) * P], pt[:])
            nc.vector.tensor_scalar_mul(x2b[j][:], x2b[j][:], c_p[:jl, j:j + 1])
        y1T = sp.tile([P, 2 * MY], F32, name="y1T")
        for m in range(2):
            pm = psX.tile([P, MY], F32, tag="py1", name=f"py1{m}", bufs=1)
            nmm = len(C2000)
            for j, (js, jl) in enumerate(C2000):
                nc.tensor.matmul(pm[:], x2b[j][:, m * P:(m + 1) * P],
                                 t0T[j][:, :], start=(j == 0),
                                 stop=(j == nmm - 1))
            nc.scalar.copy(y1T[:, m * MY:(m + 1) * MY], pm[:])
        prb = psX.tile([P, 512], F32, tag="sm", name="prb", bufs=1)
        nc.tensor.matmul(prb[:, :MY], ones_r[:], rvec[:], start=True, stop=True)
        rb = sp.tile([P, MY], F32, name="rb")
        nc.scalar.copy(rb[:], prb[:, :MY])
        for m in range(2):
            nc.vector.tensor_tensor(y1T[:, m * MY:(m + 1) * MY],
                                    y1T[:, m * MY:(m + 1) * MY], rb[:],
                                    op=ALU.mult)

        # ---- y2 partial = t0^T (r * x1_mine); reduce-scatter within group
        x1n = [sp.tile([ln, F], F32, name=f"x1n{ci}", padded_shape=[P, F])
               for ci, (cs, ln) in enumerate(C500)]
        for ci, (cs, cl) in enumerate(C500):
            for fc in range(2):
                pt = psX.tile([cl, P], F32, tag="pt", name=f"px1n{ci}{fc}",
                              padded_shape=[P, P], bufs=2)
                _tr(nc, pt[:], x1Tm[fc][:, cs:cs + cl], ident)
                nc.vector.tensor_copy(x1n[ci][:, fc * P:(fc + 1) * P], pt[:])
            nc.vector.tensor_scalar_mul(x1n[ci][:], x1n[ci][:],
                                        r_p[:cl, ci:ci + 1])
        rsi = drb.tile([N, F], F32, tag="rsi", name="rsi")
        for j, (js, jl) in enumerate(C2000):
            pj = psX.tile([P, F], F32, tag="py2", name=f"py2{j}", bufs=2)
            _mm(nc, pj[:jl, :], [(t0[ci][:, js:js + jl], x1n[ci][:])
                                 for ci in range(len(C500))])
            stg = sp.tile([jl, F], F32, tag="y2s", name=f"y2s{j}",
                          padded_shape=[P, F], bufs=3)
            nc.vector.tensor_copy(stg[:], pj[:jl, :])
            nc.sync.dma_start(rsi[js:js + jl, :], stg[:])
        rso = drb.tile([MY, F], F32, tag="rso", name="rso")
        nc.gpsimd.collective_compute(
            "ReduceScatter", ALU.add, ins=[rsi.opt()], outs=[rso.opt()],
            replica_groups=[[0, 1, 2, 3], [4, 5, 6, 7]])
        y2T = sp.tile([P, 2 * MY], F32, name="y2T")
        for ci, (cs, cl) in enumerate(C500):
            ld = sp.tile([cl, F], F32, tag="y2l", name=f"y2l{ci}",
                         padded_shape=[P, F], bufs=3)
            nc.sync.dma_start(ld[:], rso[cs:cs + cl, :])
            for fc in range(2):
                pt = psX.tile([P, cl], F32, tag="pt", name=f"py2T{ci}{fc}",
                              padded_shape=[P, P], bufs=2)
                _tr(nc, pt[:], ld[:, fc * P:(fc + 1) * P], ident)
                nc.vector.tensor_copy(y2T[:, fc * MY + cs: fc * MY + cs + cl],
                                      pt[:])
        cslice = sp.tile([1, MY], F32, name="cslice")
        nc.sync.dma_start(cslice[:],
                          pg._cvec_d[0:1, bass.ds(pg._base_row, MY)])
        pcs = psX.tile([P, 512], F32, tag="sm", name="pcs", bufs=1)
        nc.tensor.matmul(pcs[:, :MY], ones_r[:], cslice[:], start=True, stop=True)
        csb = sp.tile([P, MY], F32, name="csb")
        nc.scalar.copy(csb[:], pcs[:, :MY])
        for m in range(2):
            nc.vector.tensor_tensor(y2T[:, m * MY:(m + 1) * MY],
                                    y2T[:, m * MY:(m + 1) * MY], csb[:],
                                    op=ALU.mult)

        # ---- x <- [x, y] @ cgw  (no l2norm, no bias)
        for xm, yT, nm in ((x1Tm, y1T, "cx1"), (x2Tm, y2T, "cx2")):
            newT = sp.tile([P, 2 * MY], F32, tag="newT", name=f"{nm}T", bufs=1)
            for ci, (cs, cl) in enumerate(C500):
                px = psX.tile([P, F], F32, tag="pcx", name=f"pcx{nm}{ci}",
                              bufs=2)
                pairs = [(xm[fc][:, cs:cs + cl], cgw_t[:, fc * F:(fc + 1) * F])
                         for fc in range(2)]
                pairs += [(yT[:, fc * MY + cs: fc * MY + cs + cl],
                           cgw_t[:, (2 + fc) * F:(3 + fc) * F])
                          for fc in range(2)]
                _mm(nc, px[:cl, :], pairs)
                xn = sp.tile([cl, F], F32, tag="xnc", name=f"xnc{nm}{ci}",
                             padded_shape=[P, F], bufs=2)
                nc.vector.tensor_copy(xn[:], px[:cl, :])
                for fc in range(2):
                    pt = psX.tile([P, cl], F32, tag="pt", name=f"pxm{nm}{ci}{fc}",
                                  padded_shape=[P, P], bufs=2)
                    _tr(nc, pt[:], xn[:, fc * P:(fc + 1) * P], ident)
                    nc.vector.tensor_copy(
                        newT[:, fc * MY + cs: fc * MY + cs + cl], pt[:])
            for m in range(2):
                nc.vector.tensor_copy(xm[m][:], newT[:, m * MY:(m + 1) * MY])


# ------------------------------------------------------------------ host side
_CACHE = {}


def _get_program():
    if "pg" not in _CACHE:
        _CACHE["pg"] = build_program()
    return _CACHE["pg"]


def kernel(emb1, emb2, edge_emb1, edge_emb2, Aidx_src, Aidx_tgt,
           fc1n_w, fc1n_b, fc2n_w, fc2n_b, fc1e_w, fc1e_b, fc2e_w, fc2e_b,
           gnn_a_w, gnn_a_b, gnn_u_w, gnn_u_b, aff_A, cg_w, cg_b):
    f = np.float32
    for b in (fc1n_b, fc2n_b, fc1e_b, fc2e_b, gnn_a_b, gnn_u_b, cg_b):
        assert not np.any(np.asarray(b)), "nonzero biases not supported"
    A1 = np.asarray(Aidx_src, f)[0]
    A2 = np.asarray(Aidx_tgt, f)[0]
    assert A1.min() >= 0 and A2.min() >= 0, "negative A not supported"
    A1p = np.zeros((NT, NTP), f); A1p[:, :NT] = A1
    A2p = np.zeros((NT, NTP), f); A2p[:, :NT] = A2
    aff = np.asarray(aff_A, f)
    ssym = (aff + np.transpose(aff, (0, 2, 1))) * 0.5
    e1 = np.asarray(emb1, f)[0]; e2 = np.asarray(emb2, f)[0]
    d1 = np.asarray(edge_emb1, f)[0]; d2 = np.asarray(edge_emb2, f)[0]

    shared = {
        "emb1": e1, "emb2": e2, "eed1": d1, "eed2": d2,
        "w1n": np.asarray(fc1n_w, f), "w2n": np.asarray(fc2n_w, f),
        "w1e": np.asarray(fc1e_w, f), "w2e": np.asarray(fc2e_w, f),
        "wa": np.asarray(gnn_a_w, f), "wu": np.asarray(gnn_u_w, f),
        "ssym": ssym, "cgw": np.asarray(cg_w, f),
    }
    in_maps = []
    for c in range(NC):
        m = dict(shared)
        m["a1s"] = np.ascontiguousarray(A1p[c * MY:(c + 1) * MY])
        m["a2s"] = np.ascontiguousarray(A2p[c * MY:(c + 1) * MY])
        m["emb1m"] = np.ascontiguousarray(e1[c * KP:(c + 1) * KP])
        m["eed1m"] = np.ascontiguousarray(d1[c * KP:(c + 1) * KP])
        in_maps.append(m)

    pg = _get_program()
    res = run_bass_kernel_spmd(pg.nc, in_maps, core_ids=list(range(NC)))

    s = np.zeros((NT, NT), f)
    kp = np.zeros((N, N), f)
    ke = np.zeros((N, N), f)
    for c in range(NC):
        r = res.results[c]
        blk = (c // 4) * N
        s[c * MY:(c + 1) * MY, blk:blk + N] = r["s_out"]
        kp[c * KP:(c + 1) * KP] = r["kp"]
        ke[c * KP:(c + 1) * KP] = r["ke"]
    if DEBUG:
        _CACHE["debug"] = res.results
    return (s[None], kp[None], ke[None])
